# revision 6
# baseline (speedup 1.0000x reference)
"""BinaryLinear kernel for Trainium2, data-parallel over 8 NeuronCores.

Computes y = x @ (sign(W) * scale).T + b where
  sign(w) = +1 if w >= 0 else -1
  scale_o = max(mean_i |W[o,i]|, 1e-6)           (per output row)

Strategy
--------
- Shard batch (32768) across 8 cores -> 4096 rows/core; replicate weights.
- sign(W) and scale are computed on the HOST (scale from full-precision W,
  so that error source is gone entirely); the device only does matmuls and
  a fused scale*psum+bias epilogue.
- Mixed precision split of the 1024-long contraction, chosen so the
  measured max-rel error (1.79e-2) stays under the 2e-2 gate:
    k in [0,512):   x in fp8 e4m3, sign in fp8, matmul in DoubleRow perf
                    mode (2 fp8 weights per PE cell, rhs pair consumed at
                    2 fp8/partition/cycle -> K=256 per 512-cycle MM)
    k in [512,1024): x in bf16, sign in fp8 (exact +-1; mixed-dtype
                    lhsT fp8 x rhs bf16 runs at the full bf16 rate)
  Per (batch-block, out-block) PSUM group: 2 DR MMs + 4 bf16 MMs; the
  trace-measured steady state is a 663 ns period per {DR, bf16, bf16}
  triple = ~96.5% PE-array occupancy, i.e. the stream is at the
  accuracy-constrained PE floor (int8, which would beat e4m3 error at
  the same DR rate, is rejected by the BIR verifier; e3m4 has no
  DoubleRow; more fp8 columns breaks the 2e-2 gate at 2.5e-2).
- Block 0 runs its two DoubleRow c-sweeps FIRST (phase A: 16 DR MMs
  needing only sign+x8 fp8 tiles, 512KB of DMA), then the bf16 units in
  skewed waves (phase B).  The PE engine queue is strict FIFO, so
  without this the first bf16 MM - whose wt/xb tiles land ~3us after
  the fp8 head - stalls the whole stream behind it.  Phase A is a
  ~3.8us runway that covers the bf16 head DMA and keeps the PE busy
  through the HAM clock-gate window (continuous activity from the
  warmups -> K=8/8 early, no mid-stream re-throttle).
- Blocks 1..7 use the skewed wave schedule: MM(unit u, out-chunk c) at
  wave u+c, so the 8 PSUM banks finish staggered ~1 wave apart,
  epilogues never queue up, and bank recycling never stalls the PE.
- DMA: per-transfer throughput scales with size (64KB ~ 138 GB/s, 1MB ~
  341 GB/s) and each issue costs ~0.7us engine time + ~1.6us trigger
  latency, so inputs ride in just 12 large transfers (~4 per queue)
  over the three DMA-capable queues (sync/scalar/gpsimd), ordered by
  first need: {st(j0), x8 blk0} -> {st(j1), wt, xb blk0} -> block-1
  slabs -> the 0.75-1.5MB stage-2/3 slabs.  x/xb/wt are packed
  block-major on the host so every stage is one contiguous 2-12KB
  -per-partition slab; scale/bias are host-packed into one [128,16]
  tile (the per-element gather it replaces cost 2.3us of queue time).
- Epilogues alternate DVE / ACT per out-chunk; outputs collect in
  [128, 1024] bf16 tiles (two batch blocks) for full-rate 2KB-per-
  partition stores; the last block's per-c stores fan out over the
  three queues to shorten the kernel tail.
"""

import os
import sys
import types

for _p in ("/opt/trn_rl_repo",):
    if _p not in sys.path and os.path.isdir(_p):
        sys.path.append(_p)

import numpy as np
import ml_dtypes

import concourse.bacc as bacc
import concourse.mybir as mybir
from concourse import tile
from concourse.bass_utils import run_bass_kernel_spmd

N_CORES = 8
BATCH = 32768
SHARD = BATCH // N_CORES          # 4096 rows per core
IN = 1024
OUT = 1024
EPS = 1e-6
P = 128                           # SBUF partitions
NB = 512                          # moving free-dim per matmul
NBC = SHARD // NB                 # 8 batch blocks per core
OC = OUT // P                     # 8 output-feature chunks
K8 = 512                          # contraction columns done in fp8
JP = K8 // (2 * P)                # 2 DoubleRow k-pair units (256 each)
KB = (IN - K8) // P               # 4 bf16 k-chunk units (128 each)
NU = JP + KB                      # 6 accumulation units per group

F32 = mybir.dt.float32
BF16 = mybir.dt.bfloat16
FP8 = mybir.dt.float8e4
Alu = mybir.AluOpType
Act = mybir.ActivationFunctionType
DRMODE = mybir.MatmulPerfMode.DoubleRow

# Dummy matmuls bridge the PE from the preamble (~7us) until the first
# real tiles land (~10us): continuous PE activity into phase A keeps the
# HAM clock-gate busy-window filled so the PE un-throttles early.
WARM_SMALL = 24
WARM_BIG = 2

# batch-block DMA stages (one contiguous slab per stage per tensor)
X_STAGES = [(0, 1), (1, 2), (2, 5), (5, 8)]


def _install_trace_shim():
    """antenv.axon_hooks is absent in this image; recreate it so
    run_bass_kernel_spmd(trace=True) can capture NTFF profiles."""
    try:
        import antenv.axon_hooks  # noqa: F401
        return
    except ImportError:
        pass
    try:
        import trn_agent_boot.trn_boot as tb
        hooks = types.ModuleType("antenv.axon_hooks")
        hooks._hook = tb._ntff_profile_via_ctypes("/opt/axon/libaxon_pjrt.so")
        hooks.get_axon_ntff_profile_hook = lambda: hooks._hook
        hooks.set_axon_ntff_profile_hook = lambda h: setattr(hooks, "_hook", h)
        sys.modules["antenv.axon_hooks"] = hooks
        import concourse.bass_utils as bass_utils
        bass_utils.upload_artifacts = lambda tmpdir: f"file://{tmpdir}"
    except Exception:
        pass


def build_program():
    nc = bacc.Bacc("TRN2", target_bir_lowering=False, debug=False,
                   num_devices=N_CORES)

    # x8: fp8 part of x^T packed block-major: row p, block nb at byte
    # nb*2048, layout [j][i][nn] inside -> every stage is one contiguous
    # slab and rhs slices are [128, 2, 512] DoubleRow APs.
    x8_d = nc.dram_tensor("x8", [P, NBC * 2048], FP8, kind="ExternalInput")
    # xb: bf16 part of x^T packed [p][nb][mp][mm][nn] -> per-stage
    # contiguous slabs with 4KB rows.
    xb_d = nc.dram_tensor("xb", [P, NBC * 2048], BF16, kind="ExternalInput")
    # st: fp8 sign(W)^T for k<512, row j*128+p, cols [oh][i][o']
    st_d = nc.dram_tensor("st", [JP * P, 2 * OUT], FP8, kind="ExternalInput")
    # wt: fp8 sign(W)^T for k>=512 (+-1 exact in e4m3; moving rhs stays
    # bf16 so the matmul runs at the 1-column/cycle bf16 rate)
    wt_d = nc.dram_tensor("wt", [KB * P, OUT], FP8, kind="ExternalInput")
    # scb: host-packed scale/bias columns [p, c] / [p, OC+c]
    scb_d = nc.dram_tensor("scb", [P, 2 * OC], F32, kind="ExternalInput")
    yt_d = nc.dram_tensor("yt", [OUT, SHARD], BF16, kind="ExternalOutput")

    with tile.TileContext(nc) as tc:
        with (
            tc.tile_pool(name="w_pool", bufs=1) as w_pool,
            tc.tile_pool(name="x_pool", bufs=1) as x_pool,
            tc.tile_pool(name="misc", bufs=1) as misc,
            tc.tile_pool(name="ps", bufs=8, space="PSUM") as ps_pool,
            tc.tile_pool(name="yo_pool", bufs=8) as yo_pool,
        ):
            # ---- PE warm-up (no input deps)
            warm = misc.tile([P, NB], BF16, tag="warm", name="warm")
            nc.vector.memset(warm[:], 0.0)
            wps = ps_pool.tile([P, NB], F32, tag="ps", name="wps")
            for _ in range(WARM_SMALL):
                nc.tensor.matmul(wps[:, 0:64], warm[:, 0:P], warm[:, 0:64],
                                 start=True, stop=True)
            for _ in range(WARM_BIG):
                nc.tensor.matmul(wps[:], warm[:, 0:P], warm[:],
                                 start=True, stop=True)

            # ---- tiles
            # st per j: [P, 4(oh*2+i), 512(o')] - one 256KB 2KB-row DMA,
            # lhsT slices are clean 3D [128, 2, 128] APs.
            stj = [w_pool.tile([P, 4, OUT // 2], FP8, tag=f"st{j}",
                               name=f"st{j}") for j in range(JP)]
            # wt: all four bf16-unit sign chunks in one flat tile, one
            # 512KB DMA; lhsT slices are 2D [128, 128].
            wtall = w_pool.tile([P, KB * OUT], FP8, tag="wt", name="wt")
            # x8/xb per-stage slabs [P, (b1-b0)*2048]
            x8s = [x_pool.tile([P, (b1 - b0) * 2048], FP8,
                               tag=f"x8s_{si}", name=f"x8s_{si}")
                   for si, (b0, b1) in enumerate(X_STAGES)]
            xbs = [x_pool.tile([P, (b1 - b0) * 2048], BF16,
                               tag=f"xbs_{si}", name=f"xbs_{si}")
                   for si, (b0, b1) in enumerate(X_STAGES)]
            scb = misc.tile([P, 2 * OC], F32, tag="scb", name="scb")

            def load_x8s(si, eng):
                b0, b1 = X_STAGES[si]
                eng.dma_start(x8s[si][:], x8_d.ap()[:, b0 * 2048:b1 * 2048])

            def load_xbs(si, eng):
                b0, b1 = X_STAGES[si]
                eng.dma_start(xbs[si][:], xb_d.ap()[:, b0 * 2048:b1 * 2048])

            # ---- input DMAs: 12 large transfers, 4 per queue, ordered
            # by first need.
            nc.sync.dma_start(stj[0][:], st_d.ap()[0:P, :])          # pA w0
            load_x8s(0, nc.scalar)                                   # pA w0
            nc.gpsimd.dma_start(                                     # pB w0+
                wtall[:],
                wt_d.ap().rearrange("(m p) o -> p m o", m=KB))
            nc.sync.dma_start(stj[1][:], st_d.ap()[P:2 * P, :])      # pA u1
            load_xbs(0, nc.scalar)                                   # pB w0
            nc.gpsimd.dma_start(scb[:], scb_d.ap())                  # epi 0
            nc.sync.dma_start(xbs[1][:], xb_d.ap()[:, 2048:4096])    # blk1 pB
            load_x8s(1, nc.scalar)                                   # blk1 pA
            load_x8s(2, nc.gpsimd)                                   # blk2-4
            load_xbs(2, nc.sync)
            load_xbs(3, nc.scalar)                                   # blk5-7
            load_x8s(3, nc.gpsimd)

            def stage_of(n):
                for si, (b0, b1) in enumerate(X_STAGES):
                    if b0 <= n < b1:
                        return si, n - b0
                raise AssertionError(n)

            def rhs_for(u, n):
                si, ln = stage_of(n)
                if u < JP:
                    base = ln * 2048 + u * 1024
                    return x8s[si][:, base:base + 1024].rearrange(
                        "p (i n) -> p i n", i=2)
                m = u - JP
                base = ln * 2048 + m * NB
                return xbs[si][:, base:base + NB]

            yo_cur = [None] * OC

            def epilogue(n, c, ps):
                half = n % 2
                if half == 0:
                    yo_cur[c] = yo_pool.tile([P, 2 * NB], BF16, tag="yo",
                                             name=f"yo{n}_{c}")
                yo = yo_cur[c]
                dst = yo[:, half * NB:(half + 1) * NB]
                if c % 2 == 0:
                    nc.vector.tensor_scalar(dst, ps[:], scb[:, c:c + 1],
                                            scb[:, OC + c:OC + c + 1],
                                            Alu.mult, Alu.add)
                else:
                    nc.scalar.activation(dst, ps[:], Act.Identity,
                                         bias=scb[:, OC + c:OC + c + 1],
                                         scale=scb[:, c:c + 1])
                if n == NBC - 2:
                    # penultimate block: store its half immediately so it
                    # overlaps the last block's compute
                    nc.scalar.dma_start(
                        yt_d.ap()[c * P:(c + 1) * P, n * NB:(n + 1) * NB],
                        yo[:, 0:NB])
                elif n == NBC - 1:
                    # last block: per-c half stores fan out over the three
                    # DMA queues as each staggered epilogue completes ->
                    # short kernel tail
                    eng = (nc.sync, nc.scalar, nc.gpsimd)[c % 3]
                    eng.dma_start(
                        yt_d.ap()[c * P:(c + 1) * P, n * NB:(n + 1) * NB],
                        yo[:, NB:2 * NB])
                elif half == 1:
                    eng = nc.scalar if c % 2 == 1 else nc.sync
                    eng.dma_start(
                        yt_d.ap()[c * P:(c + 1) * P,
                                  (n - 1) * NB:(n + 1) * NB],
                        yo[:])

            def lhsT_dr(u, c):
                oh = c // 4
                return stj[u][:, oh * 2:oh * 2 + 2,
                              (c % 4) * P:(c % 4 + 1) * P]

            def lhsT_bf(s, c):
                return wtall[:, s * OUT + c * P:s * OUT + (c + 1) * P]

            # Per-bank unit order for blocks >=1: DoubleRow MMs at slots 0
            # and 3 so two DR MMs are never issued back-to-back (a DR pair
            # costs an extra ~30ns drain gap when adjacent).
            UORDER = (0, 2, 3, 1, 4, 5)

            def mm(s, c, n, ps):
                u = UORDER[s]
                if u < JP:
                    nc.tensor.matmul(ps[:], lhsT_dr(u, c),
                                     rhs_for(u, n), start=(s == 0), stop=False,
                                     perf_mode=DRMODE)
                else:
                    nc.tensor.matmul(ps[:], lhsT_bf(u - JP, c),
                                     rhs_for(u, n), start=(s == 0),
                                     stop=(s == NU - 1))

            # ---- block 0: phase A (DR c-sweeps, fp8 data only) then
            # phase B (bf16 units in skewed waves)
            yps = [ps_pool.tile([P, NB], F32, tag="ps", name=f"yp0_{c}")
                   for c in range(OC)]
            for u in range(JP):
                for c in range(OC):
                    nc.tensor.matmul(yps[c][:], lhsT_dr(u, c), rhs_for(u, 0),
                                     start=(u == 0), stop=False,
                                     perf_mode=DRMODE)
            for wv in range(KB + OC - 1):
                for c in range(OC):
                    s = wv - c
                    if 0 <= s < KB:
                        nc.tensor.matmul(
                            yps[c][:], lhsT_bf(s, c), rhs_for(JP + s, 0),
                            start=False, stop=(s == KB - 1))
                        if s == KB - 1:
                            epilogue(0, c, yps[c])

            # ---- blocks 1..7: skewed waves.  MM(unit u, out-chunk c) at
            # wave u+c; bank completions stagger ~1 wave apart.
            for n in range(1, NBC):
                yps = [ps_pool.tile([P, NB], F32, tag="ps", name=f"yp{n}_{c}")
                       for c in range(OC)]
                for wv in range(NU + OC - 1):
                    for c in range(OC):
                        u = wv - c
                        if 0 <= u < NU:
                            mm(u, c, n, yps[c])
                            if u == NU - 1:
                                epilogue(n, c, yps[c])

    nc.compile()
    return nc


_NC = None


def _get_program():
    global _NC
    if _NC is None:
        _NC = build_program()
    return _NC


def kernel(x: np.ndarray, W: np.ndarray, b: np.ndarray) -> np.ndarray:
    assert x.shape == (BATCH, IN) and W.shape == (OUT, IN) and b.shape == (OUT,)
    nc = _get_program()

    Wf = np.asarray(W, dtype=np.float32)
    sgnT = np.where(Wf >= 0, np.float32(1.0), np.float32(-1.0)).T  # [in, out]
    # st cols per j are [oh (out half)][i (k subtile)][o']
    st_pack = np.ascontiguousarray(
        sgnT[:K8].reshape(JP, 2, P, 2, OUT // 2).transpose(0, 2, 3, 1, 4)
        .reshape(JP * P, 2 * OUT)).astype(ml_dtypes.float8_e4m3)
    wt_pack = np.ascontiguousarray(sgnT[K8:]).astype(ml_dtypes.float8_e4m3)
    sc = np.maximum(np.abs(Wf).mean(axis=1), EPS).astype(np.float32)
    b32 = np.asarray(b, dtype=np.float32)
    # scb[p, c] = sc[c*128+p]; scb[p, OC+c] = b[c*128+p]
    scb = np.ascontiguousarray(
        np.concatenate([sc.reshape(OC, P).T, b32.reshape(OC, P).T],
                       axis=1).astype(np.float32))

    in_maps = []
    for c in range(N_CORES):
        xt = x[c * SHARD:(c + 1) * SHARD].T      # [in, n] view
        # x8 block-major: (j,i,p,nb,nn) -> (p, nb, j, i, nn)
        x8 = xt[:K8].astype(ml_dtypes.float8_e4m3)
        x8 = np.ascontiguousarray(
            x8.reshape(JP, 2, P, NBC, NB).transpose(2, 3, 0, 1, 4)
            .reshape(P, NBC * 2048))
        # xb block-major: (mp,mm,p,nb,nn) -> (p, nb, mp, mm, nn)
        xb = xt[K8:].astype(ml_dtypes.bfloat16)
        xb = np.ascontiguousarray(
            xb.reshape(2, 2, P, NBC, NB).transpose(2, 3, 0, 1, 4)
            .reshape(P, NBC * 2048))
        in_maps.append({"x8": x8, "xb": xb, "st": st_pack, "wt": wt_pack,
                        "scb": scb})

    trace = bool(int(os.environ.get("BINLIN_TRACE", "0")))
    if trace:
        _install_trace_shim()
    res = run_bass_kernel_spmd(nc, in_maps, core_ids=list(range(N_CORES)),
                               trace=trace)
    if trace and res.exec_time_ns is not None:
        print(f"HW exec time: {res.exec_time_ns} ns", flush=True)

    y = np.empty((BATCH, OUT), dtype=np.float32)
    for c in range(N_CORES):
        y[c * SHARD:(c + 1) * SHARD] = res.results[c]["yt"].T.astype(np.float32)
    return y
